# revision 77
# baseline (speedup 1.0000x reference)
"""Trainium2 Bass kernel for a GNN message-passing layer (BoundaryConvLayer).

Computation (reference, per node i over D=128 channels):
    rate  = softplus(x @ W_rate) + EPS
    gamma = x @ W_rob + b_rob
    h     = x @ W_fc + b_fc
    agg   = segment_sum(h[row] + h[col], row)
    y     = LayerNorm((rate*agg + gamma) / (1 + rate*deg + EPS)) * ln_gamma + ln_beta

Distribution: nodes sharded across 8 cores by contiguous row blocks; edges
partitioned by destination row so the segment sum is local.

Design (v5):
  * Per-core COMPACT gather table (phase 1): only the ~63% of nodes referenced
    as sources by this core's edges are materialized (g = x@W_fc rows, bf16),
    split into NCHK=2 chunks so dma_gather's int16 indices reach all rows.
    PSUM evacuation alternates Scalar (2/3) and Vector (1/3).
  * agg identity: agg[i] = cnt[i]*h[i] + sum_{e:row=i} h[col_e], cnt = in-edge
    count.  Neighbor sum via one-hot "selection matrix" matmuls accumulated in
    PSUM; self term via identity matmul of hlc = cnt*g_local; the fc bias
    enters as a K=1 matmul (cntT x 2*b_fc), b_rob as a broadcast DVE add.
  * Per-tile GEMMs fused: one matmul with moving rhs [W_rate|W_rob] (N=256)
    plus one for W_fc.  Eltwise is SOFTWARE-PIPELINED one batch behind the
    matmuls (in-order engines never head-of-line block): rate/gamma evac
    issue immediately at batch end (elt_pre), the rest one batch later.
    den and the final (y-mean)*rstd run as per-partition-scale/bias Scalar
    ACTs; LayerNorm stats via DVE bn_stats/bn_aggr; fast custom-DVE
    reciprocal for 1/den.
"""

import numpy as np
import ml_dtypes
from contextlib import ExitStack
from dataclasses import dataclass

import concourse.bass as bass
import concourse.tile as tile
from concourse import bacc, mybir
from concourse.bass_utils import run_bass_kernel_spmd

# The stock ACT-table chooser greedily picks the first set containing each
# function, which can alternate between sets and reload the table (~1.3us
# each).  Restrict it to the one set that contains all of {Exp, Ln, Copy}.
_ACT_KEEP = "natural_log_exp_and_others"
if not getattr(bacc, "_act_tables_patched", False):
    _orig_get_tables = bacc.get_activation_tables

    def _patched_get_tables(arch):
        t = _orig_get_tables(arch)
        if _ACT_KEEP in t:
            t = {k: (v if k == _ACT_KEEP else set()) for k, v in t.items()}
        return t

    bacc.get_activation_tables = _patched_get_tables
    bacc._act_tables_patched = True

BF16 = ml_dtypes.bfloat16
EPS = 1e-4
LN_EPS = 1e-5
P = 128
D = 128


@dataclass
class Cfg:
    N: int            # total nodes
    E: int            # total edges
    NC: int           # cores
    NCHK: int = 2     # gather table chunks (int16 range)
    CPAD: int = 32256 # rows per chunk (252*128, < 32768 for int16 idx)
    Cq: int = 0       # 128-slot groups per (tile, chunk); set by prep
    ln_trivial: bool = False

    @property
    def NLOC(self):
        return self.N // self.NC

    @property
    def T(self):
        return (self.NLOC + P - 1) // P

    @property
    def TLP(self):
        return self.T * P

    @property
    def NCOL(self):  # x-compact table columns
        return self.NCHK * self.CPAD

    @property
    def G(self):      # tiles per gather group
        for g in (7, 14, 4, 2, 1):
            if self.T % g == 0:
                return g
        return 1


def prep(x, edge_index, degree, W_fc, b_fc, W_rate, W_rob, b_rob, ln_gamma, ln_beta,
         cfg: Cfg):
    """Host-side preprocessing: shard + build per-core compact gather tables."""
    N, NC, NCHK, CPAD = cfg.N, cfg.NC, cfg.NCHK, cfg.CPAD
    NLOC, T, TLP = cfg.NLOC, cfg.T, cfg.TLP

    x = np.asarray(x, np.float32)
    edge_index = np.asarray(edge_index, np.int64)
    degree = np.asarray(degree)
    row, col = edge_index[0], edge_index[1]
    xT = x.T.astype(BF16)  # [D, N]

    w_fc = np.ascontiguousarray(W_fc, dtype=np.float32).astype(BF16)
    w_rt = np.ascontiguousarray(W_rate, dtype=np.float32).astype(BF16)
    w_rb = np.ascontiguousarray(W_rob, dtype=np.float32).astype(BF16)
    wcat = np.concatenate([w_rt, w_rb], axis=1)          # [128, 256]
    # b_rob is folded into the eltwise as a broadcast add (brobF, replicated
    # across partitions); no bias matmul needed for the rate|gamma GEMM.
    brobF = np.broadcast_to(np.asarray(b_rob, np.float32)[None, :],
                            (P, D)).astype(BF16).copy()
    # table is bias-free (g = x@W_fc); the self-term chain adds cnt*2bfc via
    # a K=1 matmul (cntT x bfc2), so
    # cnt*g_i + cnt*2bfc + sum_slots g[col] == cnt*h_i + sum h[col] exactly
    bfc2 = 2.0 * np.asarray(b_fc, np.float32).reshape(1, D)
    onesr = np.ones((1, P), np.float32)

    cfg.ln_trivial = bool(np.all(np.asarray(ln_gamma) == 1.0)
                          and np.all(np.asarray(ln_beta) == 0.0))
    lnab = np.zeros((P, 2 * D), np.float32)
    lnab[:, :D] = np.asarray(ln_gamma, np.float32)[None, :]
    lnab[:, D:] = np.asarray(ln_beta, np.float32)[None, :]

    core_of = row // NLOC
    CELL = 4 * P          # target per-(tile,chunk) occupancy for Cq=4

    # pass 1: per-core tile balancing (permute local nodes so every tile has
    # <=128 nodes and ~<=2*CELL edges) + greedy source 2-coloring so each
    # (tile, chunk) cell stays <= CELL.  If a core misses, Cq grows to 5 and
    # the program adapts (capacity is computed from the achieved maximum).
    import heapq
    percore = []
    maxslots = 0
    cfg.newpos = []
    for r in range(NC):
        m = core_of == r
        rl0 = row[m] - r * NLOC
        ce = col[m]

        # -- node -> tile assignment (LPT greedy on edge count, <=128 nodes)
        cnt_node = np.bincount(rl0, minlength=NLOC)
        order_n = np.argsort(-cnt_node, kind="stable")
        heap = [(0, t) for t in range(T)]
        heapq.heapify(heap)
        nslots = np.zeros(T, np.int64)
        tile_of = np.zeros(NLOC, np.int64)
        for nid in order_n:
            c = cnt_node[nid]
            while True:
                load, t = heapq.heappop(heap)
                if nslots[t] < P:
                    break  # full tiles never take nodes again: drop them
            tile_of[nid] = t
            nslots[t] += 1
            heapq.heappush(heap, (load + int(c), t))
        # slot within tile
        slot_in = np.zeros(NLOC, np.int64)
        fill = np.zeros(T, np.int64)
        for nid in np.argsort(tile_of, kind="stable"):
            t = tile_of[nid]
            slot_in[nid] = fill[t]
            fill[t] += 1
        newpos = tile_of * P + slot_in          # orig local id -> device row
        cfg.newpos.append(newpos)
        rl = newpos[rl0]
        t_e = rl // P

        # -- source chunk 2-coloring
        uniq, cid = np.unique(ce, return_inverse=True)
        NU = len(uniq)
        ut, ut_cnt = np.unique(cid * T + t_e, return_counts=True)
        u_of = ut // T
        t_of = ut % T
        tot = np.bincount(u_of, weights=ut_cnt, minlength=NU).astype(np.int64)
        starts = np.searchsorted(u_of, np.arange(NU + 1))
        loads = np.zeros((T, NCHK), np.int64)
        color = np.full(NU, -1, np.int64)
        csize = np.zeros(NCHK, np.int64)
        multi = np.where(tot > 1)[0]
        for u in multi[np.argsort(-tot[multi], kind="stable")]:
            s, e = starts[u], starts[u + 1]
            ts, cs = t_of[s:e], ut_cnt[s:e]
            best, bestkey = 0, None
            for c in range(NCHK):
                over = np.maximum(loads[ts, c] + cs - CELL, 0).sum()
                key = (over, int(np.max(loads[ts, c] + cs)), csize[c])
                if bestkey is None or key < bestkey:
                    best, bestkey = c, key
            color[u] = best
            loads[ts, best] += cs
            csize[best] += 1
        singles = np.where(tot == 1)[0]
        st_t = t_of[starts[singles]]
        for t in range(T):
            su = singles[st_t == t]
            k = len(su)
            if k == 0:
                continue
            l0, l1 = loads[t, 0], loads[t, 1]
            n0 = int(np.clip((k + l1 - l0 + 1) // 2, 0, k))
            color[su[:n0]] = 0
            color[su[n0:]] = 1
            loads[t, 0] += n0
            loads[t, 1] += k - n0
            csize[0] += n0
            csize[1] += k - n0
        assert (color >= 0).all()
        assert csize.max() <= CPAD, (r, csize)
        # row within chunk, in ascending-uniq order (gather locality)
        rowin_u = np.zeros(NU, np.int64)
        for c in range(NCHK):
            sel_u = color == c
            rowin_u[sel_u] = np.arange(int(sel_u.sum()))
        q_e = color[cid]
        rowin_e = rowin_u[cid]
        cnt_tq = np.bincount(t_e * NCHK + q_e, minlength=T * NCHK).reshape(T, NCHK)
        maxslots = max(maxslots, int(cnt_tq.max()))
        percore.append((rl, uniq, q_e, rowin_e, t_e, cnt_tq, color, rowin_u))
    Cq = max(1, -(-maxslots // P))
    cfg.Cq = Cq
    G = cfg.G
    NG = T // G
    IPG = G * Cq * P

    in_maps = []
    for r in range(NC):
        rl, uniq, q_e, rowin_e, t_e, cnt_tq, color, rowin_u = percore[r]
        newpos = cfg.newpos[r]

        # x-compact: chunk q of the table holds source u at column
        # q*CPAD + rowin_u; unused tail columns stay zero.
        xTc = np.zeros((P, NCHK * CPAD), BF16)
        xTc[:, color * CPAD + rowin_u] = xT[:, uniq]

        # order edges by (tile, chunk, SOURCE row) so each gather run reads
        # ascending addresses (HBM row-buffer locality)
        order = np.lexsort((rowin_e, q_e, t_e))
        rl_s, q_s, rw_s, t_s = rl[order], q_e[order], rowin_e[order], t_e[order]
        tq_s = t_s * NCHK + q_s
        run_start = np.zeros(T * NCHK + 1, np.int64)
        np.cumsum(cnt_tq.reshape(-1), out=run_start[1:])
        pos = np.arange(len(rl_s)) - run_start[tq_s]
        tl_s = t_s % G
        gg_s = t_s // G
        ipos = tl_s * (Cq * P) + pos
        idx16 = np.zeros((NCHK, NG, IPG), np.int16)  # pad -> row 0 (sel kills it)
        idx16[q_s, gg_s, ipos] = rw_s.astype(np.int16)
        # wrap each stream: idx i -> [i%16, i//16], replicate to 128 partitions
        idxw = idx16.reshape(NCHK, NG, IPG // 16, 16).transpose(0, 1, 3, 2)
        idxw = np.ascontiguousarray(idxw)
        idxw = np.tile(idxw, (1, 1, 8, 1))           # [NCHK, NG, 128, IPG//16]
        idx_sb = np.ascontiguousarray(
            idxw.transpose(2, 0, 1, 3)).reshape(P, NCHK * NG * (IPG // 16))

        # rowsr: rebased dst row (node % 128) per slot, -1 for pads
        rowsr = np.full((P, T * NCHK * Cq), -1.0, BF16)
        slot_col = t_s * (NCHK * Cq) + q_s * Cq + pos // P
        rowsr[pos % P, slot_col] = (rl_s % P).astype(BF16)

        iotab = np.broadcast_to(
            np.tile(np.arange(P, dtype=BF16)[None, :], (1, NCHK * Cq)),
            (P, NCHK * Cq * P)).copy()

        cnt = np.bincount(rl, minlength=TLP)
        cntb = cnt.astype(np.float32).reshape(T, P).T.copy()
        cntT = cnt.astype(np.float32).reshape(1, TLP)
        degl = np.zeros(TLP, np.float32)
        degl[newpos] = degree[r * NLOC:(r + 1) * NLOC].astype(np.float32)
        degf = degl.reshape(T, P).T.copy()
        degb2 = 1.0 + EPS + EPS * degf
        xTloc = np.zeros((P, TLP), BF16)
        xTloc[:, newpos] = xT[:, r * NLOC:(r + 1) * NLOC]

        in_maps.append({
            "xTc": xTc, "xTloc": xTloc,
            "Wfc": w_fc, "Wcat": wcat,
            "brobF": brobF, "bfc2": bfc2, "lnab": lnab,
            "iotab": iotab, "rowsr": rowsr, "idxs": idx_sb,
            "cntb": cntb, "cntT": cntT, "degf": degf, "degb2": degb2,
            "ident": np.eye(P, dtype=BF16),
        })
    return in_maps


def build(cfg: Cfg):
    """Build the SPMD Bass program (identical on every core)."""
    NC, T, TLP = cfg.NC, cfg.T, cfg.TLP
    NCHK, Cq, CPAD, NCOL = cfg.NCHK, cfg.Cq, cfg.CPAD, cfg.NCOL
    G = cfg.G
    NG = T // G
    IPG = G * Cq * P
    SELW = NCHK * Cq * P       # sel width per tile
    bf = mybir.dt.bfloat16
    f32 = mybir.dt.float32
    f8 = mybir.dt.float8e4
    i16 = mybir.dt.int16
    B = 4                      # tiles per eltwise batch

    nc = bacc.Bacc("TRN2", target_bir_lowering=False, debug=False, num_devices=NC,
                   num_swdge_queues=4)
    # pre-create ACT bias consts so no memsets land mid-loop
    for val in (LN_EPS, 0.0, 1.0):
        if (f32, val) in nc.const_aps.aps:
            continue
        cs = nc.alloc_sbuf_tensor(f"const-float32-{val}", [P, 1], f32)
        nc.gpsimd.memset(cs.ap(), val)
        nc.const_aps.aps[(f32, val)] = cs.ap()
    nc.all_engine_barrier()

    d_xTc = nc.dram_tensor("xTc", [P, NCOL], bf, kind="ExternalInput").ap()
    d_xTloc = nc.dram_tensor("xTloc", [P, TLP], bf, kind="ExternalInput").ap()
    d_wfc = nc.dram_tensor("Wfc", [P, D], bf, kind="ExternalInput").ap()
    d_wcat = nc.dram_tensor("Wcat", [P, 2 * D], bf, kind="ExternalInput").ap()
    d_brobF = nc.dram_tensor("brobF", [P, D], bf, kind="ExternalInput").ap()
    d_bfc2 = nc.dram_tensor("bfc2", [1, D], f32, kind="ExternalInput").ap()
    d_lnab = nc.dram_tensor("lnab", [P, 2 * D], f32, kind="ExternalInput").ap()
    d_iota = nc.dram_tensor("iotab", [P, SELW], bf, kind="ExternalInput").ap()
    d_rowsr = nc.dram_tensor("rowsr", [P, T * NCHK * Cq], bf,
                             kind="ExternalInput").ap()
    d_idxs = nc.dram_tensor("idxs", [P, NCHK * NG * (IPG // 16)], i16,
                            kind="ExternalInput").ap()
    d_cntb = nc.dram_tensor("cntb", [P, T], f32, kind="ExternalInput").ap()
    d_cntT = nc.dram_tensor("cntT", [1, TLP], f32, kind="ExternalInput").ap()
    d_ident = nc.dram_tensor("ident", [P, P], bf, kind="ExternalInput").ap()
    d_degf = nc.dram_tensor("degf", [P, T], f32, kind="ExternalInput").ap()
    d_degb2 = nc.dram_tensor("degb2", [P, T], f32, kind="ExternalInput").ap()
    # one h-table tensor per chunk so chunk-q gathers depend only on chunk-q
    # phase-1 writes
    d_hq = [nc.dram_tensor(f"htab{q}", [CPAD, D], bf, kind="Internal").ap()
            for q in range(NCHK)]
    d_y = nc.dram_tensor("y", [TLP, D], f32, kind="ExternalOutput").ap()

    with tile.TileContext(nc) as tc, ExitStack() as ctx:
        from concourse import library_config
        nc.gpsimd.load_library(library_config.mlp)

        # ------------- consts: load FIRST, on the Scalar queue -------------
        consts = ctx.enter_context(tc.tile_pool(name="consts", bufs=1))
        wfc = consts.tile([P, D], bf)
        nc.scalar.dma_start(wfc[:], d_wfc[:])
        wcat = consts.tile([P, 2 * D], bf)
        nc.scalar.dma_start(wcat[:], d_wcat[:])
        brobF = consts.tile([P, D], bf)
        nc.scalar.dma_start(brobF[:], d_brobF[:])
        bfc2r = consts.tile([1, D], f32)
        nc.scalar.dma_start(bfc2r[:], d_bfc2[:])
        # phase-3-only consts go on the gpsimd DMA queue (idle until the
        # gathers) so they don't delay phase-1 table writes on scalar's queue
        iota = consts.tile([P, SELW], bf)
        nc.gpsimd.dma_start(iota[:], d_iota[:])
        rowsr = consts.tile([P, T * NCHK * Cq], bf)
        nc.gpsimd.dma_start(rowsr[:], d_rowsr[:])
        idxs = consts.tile([P, NCHK * NG * (IPG // 16)], i16)
        nc.gpsimd.dma_start(idxs[:], d_idxs[:])
        cntb = consts.tile([P, T], f32)
        nc.scalar.dma_start(cntb[:], d_cntb[:])
        cntT = consts.tile([1, TLP], f32)
        nc.scalar.dma_start(cntT[:], d_cntT[:])
        degf = consts.tile([P, T], f32)
        nc.scalar.dma_start(degf[:], d_degf[:])
        degb2 = consts.tile([P, T], f32)
        nc.scalar.dma_start(degb2[:], d_degb2[:])
        xloc = consts.tile([P, TLP], bf)
        nc.gpsimd.dma_start(xloc[:], d_xTloc[:])
        ident = consts.tile([P, P], bf)
        nc.gpsimd.dma_start(ident[:], d_ident[:])
        lnab = None
        if not cfg.ln_trivial:
            lnab = consts.tile([P, 2 * D], f32)
            nc.scalar.dma_start(lnab[:], d_lnab[:])

        # ---------------- phase 1: g = x @ W_fc table ----------------
        CHUNK = 8192
        STG = 2048
        GRP = 512
        with tc.tile_pool(name="p1x", bufs=3) as p1x, \
             tc.tile_pool(name="p1ps", bufs=4, space="PSUM") as p1ps, \
             tc.tile_pool(name="p1st", bufs=4) as p1st:
            evac = 0
            for q in range(NCHK):
                for c0 in range(0, CPAD, CHUNK):
                    cw = min(CHUNK, CPAD - c0)
                    xc = p1x.tile([P, CHUNK], bf, tag="xc", name="xc")
                    nc.sync.dma_start(xc[:, :cw],
                                      d_xTc[:, q * CPAD + c0:q * CPAD + c0 + cw])
                    for g0 in range(0, cw, STG):
                        gw = min(STG, cw - g0)
                        gst = p1st.tile([P, STG], bf, tag="gst", name="gst")
                        for s in range(0, gw, GRP):
                            gps = p1ps.tile([P, GRP], f32, space="PSUM",
                                            tag="gps", name="gps")
                            for j in range(0, GRP, P):
                                nc.tensor.matmul(
                                    out=gps[:, j:j + P],
                                    lhsT=xc[:, g0 + s + j:g0 + s + j + P],
                                    rhs=wfc[:],
                                    start=True, stop=True,
                                )
                            if evac % 3 == 2:
                                nc.vector.tensor_scalar_mul(
                                    out=gst[:, s:s + GRP], in0=gps[:],
                                    scalar1=1.0)
                            else:
                                nc.scalar.copy(gst[:, s:s + GRP], gps[:])
                            evac += 1
                        dst = d_hq[q][c0 + g0:c0 + g0 + gw, :].rearrange(
                            "(t p) d -> p t d", p=P)
                        nc.scalar.dma_start(dst, gst[:, :gw].rearrange(
                            "p (t d) -> p t d", d=D))

        # ---------------- phase 3: message passing + elementwise -------------
        msg0p = ctx.enter_context(tc.tile_pool(name="msg0p", bufs=6))
        msg1p = ctx.enter_context(tc.tile_pool(name="msg1p", bufs=2))
        selp = ctx.enter_context(tc.tile_pool(name="selp", bufs=5))
        eltp = ctx.enter_context(tc.tile_pool(name="eltp", bufs=2))
        smallp = ctx.enter_context(tc.tile_pool(name="smallp", bufs=2))
        apsp = ctx.enter_context(tc.tile_pool(name="apsp", bufs=2, space="PSUM"))
        rgp = ctx.enter_context(tc.tile_pool(name="rgp", bufs=2, space="PSUM"))
        hp = ctx.enter_context(tc.tile_pool(name="hp", bufs=2, space="PSUM"))

        def elt_pre(bt, tiles):
            """Immediate part: rate + gamma evac on Scalar (inputs ready as
            soon as the batch's rg matmuls retire)."""
            nb = len(tiles)
            rg4, aps4, h4 = bt
            spt = eltp.tile([P, B, D], bf, tag="spt", name="spt")
            rate = eltp.tile([P, B, D], bf, tag="rate", name="rate")
            gamb = eltp.tile([P, B, D], bf, tag="gamb", name="gamb")
            # rate = softplus(x@Wrt) = ln(exp(z)+1); exp/ln share one ACT table
            nc.scalar.activation(out=spt[:, :nb, :], in_=rg4[:, :nb, 0:D],
                                 func=mybir.ActivationFunctionType.Exp)
            nc.scalar.activation(out=rate[:, :nb, :], in_=spt[:, :nb, :],
                                 func=mybir.ActivationFunctionType.Ln,
                                 bias=1.0)
            nc.scalar.copy(gamb[:, :nb, :], rg4[:, :nb, D:2 * D])
            return rate, gamb

        def eltwise(bt, tiles, rate, gamb):
            nb = len(tiles)
            t0g = tiles[0]
            rg4, aps4, h4 = bt
            aggb = eltp.tile([P, B, D], bf, tag="aggb", name="aggb")
            num = eltp.tile([P, B, D], bf, tag="num", name="num")
            den = eltp.tile([P, B, D], f32, tag="den", name="den")
            y0 = eltp.tile([P, B, D], f32, tag="y0", name="y0")
            yf = eltp.tile([P, B, D], f32, tag="yf", name="yf")
            st6 = smallp.tile([P, B, 6], f32, tag="st6", name="st6")
            mv = smallp.tile([P, B, 2], f32, tag="mv", name="mv")
            sm = smallp.tile([P, 2 * B], f32, tag="sm", name="sm")
            lv = sm[:, 0:nb]
            rstd = sm[:, B:B + nb]

            # evacuate agg PSUM -> bf16 SBUF on the Scalar engine
            nc.scalar.copy(aggb[:, :nb, :], aps4[:, :nb, :])
            # den = (rate+EPS)*deg + 1 + EPS, per-partition scale/bias per tile
            for jj in range(nb):
                t = t0g + jj
                nc.scalar.activation(
                    out=den[:, jj, :], in_=rate[:, jj, :],
                    func=mybir.ActivationFunctionType.Identity,
                    scale=degf[:, t:t + 1], bias=degb2[:, t:t + 1])
            nc.vector.reciprocal_approx_fast(out=den[:, :nb, :],
                                             in_=den[:, :nb, :])
            # num = (rate+EPS)*agg + gamma + b_rob
            nc.vector.scalar_tensor_tensor(
                out=num[:, :nb, :], in0=rate[:, :nb, :], scalar=EPS,
                in1=aggb[:, :nb, :],
                op0=mybir.AluOpType.add, op1=mybir.AluOpType.mult)
            nc.vector.tensor_add(out=num[:, :nb, :], in0=num[:, :nb, :],
                                 in1=gamb[:, :nb, :])
            brob_b = brobF[:, None, :].to_broadcast([P, nb, D])
            nc.vector.tensor_add(out=num[:, :nb, :], in0=num[:, :nb, :],
                                 in1=brob_b)
            nc.vector.tensor_mul(out=y0[:, :nb, :], in0=num[:, :nb, :],
                                 in1=den[:, :nb, :])
            # LayerNorm stats via bn_stats/bn_aggr -> (mean, var) per tile
            for jj in range(nb):
                nc.vector.bn_stats(out=st6[:, jj, :], in_=y0[:, jj, :])
                nc.vector.bn_aggr(out=mv[:, jj, :], in_=st6[:, jj, :])
            # rstd = (var+eps)^-0.5 = exp(-0.5*ln(var+eps))
            nc.scalar.activation(out=lv, in_=mv[:, :nb, 1],
                                 func=mybir.ActivationFunctionType.Ln,
                                 bias=LN_EPS)
            nc.scalar.activation(out=rstd, in_=lv,
                                 func=mybir.ActivationFunctionType.Exp,
                                 scale=-0.5)
            # nb2 = -mean*rstd; yf = y0*rstd + nb2 (per-partition ACT)
            nb2 = smallp.tile([P, B], f32, tag="nb2", name="nb2")
            nc.vector.scalar_tensor_tensor(
                out=nb2[:, :nb], in0=mv[:, :nb, 0], scalar=-1.0,
                in1=rstd,
                op0=mybir.AluOpType.mult, op1=mybir.AluOpType.mult)
            for jj in range(nb):
                nc.scalar.activation(
                    out=yf[:, jj, :], in_=y0[:, jj, :],
                    func=mybir.ActivationFunctionType.Identity,
                    scale=rstd[:, jj:jj + 1], bias=nb2[:, jj:jj + 1])
            if lnab is not None:
                yf3 = yf[:, :nb, :]
                lg = lnab[:, 0:D][:, None, :].to_broadcast([P, nb, D])
                lb = lnab[:, D:2 * D][:, None, :].to_broadcast([P, nb, D])
                nc.vector.tensor_mul(out=yf3, in0=yf3, in1=lg)
                nc.vector.tensor_add(out=yf3, in0=yf3, in1=lb)
            # y rows permuted to (p*T + t) so each partition writes one
            # contiguous 512B*nb stripe; unshard() undoes the permutation
            dst = d_y[:].rearrange("(p t) d -> p t d", p=P)[:, t0g:t0g + nb, :]
            nc.sync.dma_start(dst, yf[:, :nb, :])

        bt = None
        pending = None
        GCq = G * Cq
        half = [(0, (GCq + 1) // 2), ((GCq + 1) // 2, GCq // 2)]

        def issue_gather(q, gg):
            pool = msg0p if q == 0 else msg1p
            mq = pool.tile([P, GCq * D], bf, tag=f"msg{q}", name=f"msg{q}")
            icol = (q * NG + gg) * (IPG // 16)
            for si, (s0, ns) in enumerate(half):
                sec = mq[:, s0 * D:(s0 + ns) * D]
                nc.gpsimd.dma_gather(
                    out_ap=sec.rearrange("p (s d) -> p s d", d=D),
                    in_ap=d_hq[q][:],
                    idxs_ap=idxs[:, icol + s0 * 8:icol + (s0 + ns) * 8],
                    num_idxs=ns * P,
                    num_idxs_reg=ns * P,
                    elem_size=D,
                    single_packet=False,
                    queue_num=(2 * gg + si + 2 * q) % 4,
                )
            return mq

        # pre-issue the first PRE groups' chunk-0 gathers so they run during
        # the phase-1 tail (chunk 0 is written first) instead of idling until
        # chunk 1 lands
        PRE = 5
        msg0 = [None] * NG
        for g in range(min(PRE, NG)):
            msg0[g] = issue_gather(0, g)
        for gg in range(NG):
            tg0 = gg * G
            if gg + PRE < NG:
                msg0[gg + PRE] = issue_gather(0, gg + PRE)
            msgc = [msg0[gg], issue_gather(1, gg)]
            for tl in range(G):
                t = tg0 + tl
                j = t % B
                if j == 0:
                    bt = (rgp.tile([P, B, 2 * D], f32, space="PSUM", tag="rg4",
                                   name="rg4"),
                          apsp.tile([P, B, D], f32, space="PSUM", tag="aps4",
                                    name="aps4"),
                          hp.tile([P, B, D], f32, space="PSUM", tag="h4",
                                  name="h4"))

                rg4, aps4, h4 = bt
                sel = selp.tile([P, SELW], bf, tag="sel", name="sel")
                rb = rowsr[:, t * NCHK * Cq:(t + 1) * NCHK * Cq][:, :, None] \
                    .to_broadcast([P, NCHK * Cq, P])
                nc.vector.tensor_tensor(
                    out=sel.rearrange("p (c m) -> p c m", c=NCHK * Cq), in0=rb,
                    in1=iota.rearrange("p (c m) -> p c m", c=NCHK * Cq),
                    op=mybir.AluOpType.is_equal)
                # fused rate|gamma GEMM (moving N=256)
                nc.tensor.matmul(out=rg4[:, j, :],
                                 lhsT=xloc[:, t * P:(t + 1) * P],
                                 rhs=wcat[:], start=True, stop=True)
                # local h rows for the self term (bias-free)
                nc.tensor.matmul(out=h4[:, j, :],
                                 lhsT=xloc[:, t * P:(t + 1) * P],
                                 rhs=wfc[:], start=True, stop=True)
                hlc = selp.tile([P, D], bf, tag="hlc", name="hlc")
                nc.scalar.activation(out=hlc[:], in_=h4[:, j, :],
                                     func=mybir.ActivationFunctionType.Copy,
                                     scale=cntb[:, t:t + 1])
                # aps = ident@hlc + cntT x 2bfc + sum_sel sel@msg
                nc.tensor.matmul(out=aps4[:, j, :], lhsT=ident[:], rhs=hlc[:],
                                 start=True, stop=False)
                nc.tensor.matmul(out=aps4[:, j, :],
                                 lhsT=cntT[0:1, t * P:(t + 1) * P],
                                 rhs=bfc2r[0:1, :], start=False, stop=False)
                for q in range(NCHK):
                    for c in range(Cq):
                        cc = q * Cq + c
                        moff = (tl * Cq + c) * D
                        nc.tensor.matmul(
                            out=aps4[:, j, :], lhsT=sel[:, cc * P:(cc + 1) * P],
                            rhs=msgc[q][:, moff:moff + D],
                            start=False, stop=(cc == NCHK * Cq - 1))
                if j == B - 1 or t == T - 1:
                    tiles = list(range(t - j, t + 1))
                    rate, gamb = elt_pre(bt, tiles)
                    if pending is not None:
                        eltwise(*pending)
                    pending = (bt, tiles, rate, gamb)
        if pending is not None:
            eltwise(*pending)

    nc.compile()
    return nc


def unshard(res, cfg: Cfg):
    """Gather per-core outputs back to the full array, undoing the y-row
    permutation (row p*T+t holds node t*128+p) and the node->tile shuffle."""
    T = cfg.T
    ys = []
    for r in range(cfg.NC):
        yd = res.results[r]["y"].reshape(P, T, D).transpose(1, 0, 2)
        ys.append(yd.reshape(cfg.TLP, D)[cfg.newpos[r]])
    return np.concatenate(ys, axis=0)


def run(inputs, cfg: Cfg, core_ids=None):
    in_maps = prep(**inputs, cfg=cfg)
    nc = build(cfg)
    res = run_bass_kernel_spmd(nc, in_maps, core_ids=core_ids or list(range(cfg.NC)))
    return unshard(res, cfg)


def kernel(**inputs):
    cfg = Cfg(N=100_000, E=800_000, NC=8)
    return run(inputs, cfg)


# revision 80
# speedup vs baseline: 1.5777x; 1.5777x over previous
"""Trainium2 Bass kernel for a GNN message-passing layer (BoundaryConvLayer).

Computation (reference, per node i over D=128 channels):
    rate  = softplus(x @ W_rate) + EPS
    gamma = x @ W_rob + b_rob
    h     = x @ W_fc + b_fc
    agg   = segment_sum(h[row] + h[col], row)
    y     = LayerNorm((rate*agg + gamma) / (1 + rate*deg + EPS)) * ln_gamma + ln_beta

Distribution: nodes sharded across 8 cores by contiguous row blocks; edges
partitioned by destination row so the segment sum is local.

Design (v5):
  * Per-core COMPACT gather table (phase 1): only the ~63% of nodes referenced
    as sources by this core's edges are materialized (g = x@W_fc rows, bf16),
    split into NCHK=2 chunks so dma_gather's int16 indices reach all rows.
    PSUM evacuation alternates Scalar (2/3) and Vector (1/3).
  * agg identity: agg[i] = cnt[i]*h[i] + sum_{e:row=i} h[col_e], cnt = in-edge
    count.  Neighbor sum via one-hot "selection matrix" matmuls accumulated in
    PSUM; self term via identity matmul of hlc = cnt*g_local; the fc bias
    enters as a K=1 matmul (cntT x 2*b_fc), b_rob as a broadcast DVE add.
  * Per-tile GEMMs fused: one matmul with moving rhs [W_rate|W_rob] (N=256)
    plus one for W_fc.  Eltwise is SOFTWARE-PIPELINED one batch behind the
    matmuls (in-order engines never head-of-line block): rate/gamma evac
    issue immediately at batch end (elt_pre), the rest one batch later.
    den and the final (y-mean)*rstd run as per-partition-scale/bias Scalar
    ACTs; LayerNorm stats via DVE bn_stats/bn_aggr; fast custom-DVE
    reciprocal for 1/den.
"""

import numpy as np
import ml_dtypes
from contextlib import ExitStack
from dataclasses import dataclass

import concourse.bass as bass
import concourse.tile as tile
from concourse import bacc, mybir
from concourse.bass_utils import run_bass_kernel_spmd

# The stock ACT-table chooser greedily picks the first set containing each
# function, which can alternate between sets and reload the table (~1.3us
# each).  Restrict it to the one set that contains all of {Exp, Ln, Copy}.
_ACT_KEEP = "natural_log_exp_and_others"
if not getattr(bacc, "_act_tables_patched", False):
    _orig_get_tables = bacc.get_activation_tables

    def _patched_get_tables(arch):
        t = _orig_get_tables(arch)
        if _ACT_KEEP in t:
            t = {k: (v if k == _ACT_KEEP else set()) for k, v in t.items()}
        return t

    bacc.get_activation_tables = _patched_get_tables
    bacc._act_tables_patched = True

BF16 = ml_dtypes.bfloat16
EPS = 1e-4
LN_EPS = 1e-5
P = 128
D = 128


@dataclass
class Cfg:
    N: int            # total nodes
    E: int            # total edges
    NC: int           # cores
    NCHK: int = 2     # gather table chunks (int16 range)
    CPAD: int = 32256 # rows per chunk (252*128, < 32768 for int16 idx)
    Cq: int = 0       # 128-slot groups per (tile, chunk); set by prep
    ln_trivial: bool = False

    @property
    def NLOC(self):
        return self.N // self.NC

    @property
    def T(self):
        return (self.NLOC + P - 1) // P

    @property
    def TLP(self):
        return self.T * P

    @property
    def NCOL(self):  # x-compact table columns
        return self.NCHK * self.CPAD

    @property
    def G(self):      # tiles per gather group
        for g in (7, 14, 4, 2, 1):
            if self.T % g == 0:
                return g
        return 1


def prep(x, edge_index, degree, W_fc, b_fc, W_rate, W_rob, b_rob, ln_gamma, ln_beta,
         cfg: Cfg):
    """Host-side preprocessing: shard + build per-core compact gather tables."""
    N, NC, NCHK, CPAD = cfg.N, cfg.NC, cfg.NCHK, cfg.CPAD
    NLOC, T, TLP = cfg.NLOC, cfg.T, cfg.TLP

    x = np.asarray(x, np.float32)
    edge_index = np.asarray(edge_index, np.int64)
    degree = np.asarray(degree)
    row, col = edge_index[0], edge_index[1]
    xT = x.T.astype(BF16)  # [D, N]

    w_fc = np.ascontiguousarray(W_fc, dtype=np.float32).astype(BF16)
    w_rt = np.ascontiguousarray(W_rate, dtype=np.float32).astype(BF16)
    w_rb = np.ascontiguousarray(W_rob, dtype=np.float32).astype(BF16)
    wcat = np.concatenate([w_rt, w_rb], axis=1)          # [128, 256]
    # b_rob is folded into the eltwise as a broadcast add (brobF, replicated
    # across partitions); no bias matmul needed for the rate|gamma GEMM.
    brobF = np.broadcast_to(np.asarray(b_rob, np.float32)[None, :],
                            (P, D)).astype(BF16).copy()
    # table is bias-free (g = x@W_fc); the self-term chain adds cnt*2bfc via
    # a K=1 matmul (cntT x bfc2), so
    # cnt*g_i + cnt*2bfc + sum_slots g[col] == cnt*h_i + sum h[col] exactly
    bfc2 = 2.0 * np.asarray(b_fc, np.float32).reshape(1, D)
    onesr = np.ones((1, P), np.float32)

    cfg.ln_trivial = bool(np.all(np.asarray(ln_gamma) == 1.0)
                          and np.all(np.asarray(ln_beta) == 0.0))
    lnab = np.zeros((P, 2 * D), np.float32)
    lnab[:, :D] = np.asarray(ln_gamma, np.float32)[None, :]
    lnab[:, D:] = np.asarray(ln_beta, np.float32)[None, :]

    core_of = row // NLOC
    CELL = 4 * P          # target per-(tile,chunk) occupancy for Cq=4

    # pass 1: per-core tile balancing (permute local nodes so every tile has
    # <=128 nodes and ~<=2*CELL edges) + greedy source 2-coloring so each
    # (tile, chunk) cell stays <= CELL.  If a core misses, Cq grows to 5 and
    # the program adapts (capacity is computed from the achieved maximum).
    import heapq
    percore = []
    maxslots = 0
    cfg.newpos = []
    for r in range(NC):
        m = core_of == r
        rl0 = row[m] - r * NLOC
        ce = col[m]

        # -- node -> tile assignment (LPT greedy on edge count, <=128 nodes)
        cnt_node = np.bincount(rl0, minlength=NLOC)
        order_n = np.argsort(-cnt_node, kind="stable")
        heap = [(0, t) for t in range(T)]
        heapq.heapify(heap)
        nslots = np.zeros(T, np.int64)
        tile_of = np.zeros(NLOC, np.int64)
        for nid in order_n:
            c = cnt_node[nid]
            while True:
                load, t = heapq.heappop(heap)
                if nslots[t] < P:
                    break  # full tiles never take nodes again: drop them
            tile_of[nid] = t
            nslots[t] += 1
            heapq.heappush(heap, (load + int(c), t))
        # slot within tile
        slot_in = np.zeros(NLOC, np.int64)
        fill = np.zeros(T, np.int64)
        for nid in np.argsort(tile_of, kind="stable"):
            t = tile_of[nid]
            slot_in[nid] = fill[t]
            fill[t] += 1
        newpos = tile_of * P + slot_in          # orig local id -> device row
        cfg.newpos.append(newpos)
        rl = newpos[rl0]
        t_e = rl // P

        # -- source chunk 2-coloring
        uniq, cid = np.unique(ce, return_inverse=True)
        NU = len(uniq)
        ut, ut_cnt = np.unique(cid * T + t_e, return_counts=True)
        u_of = ut // T
        t_of = ut % T
        tot = np.bincount(u_of, weights=ut_cnt, minlength=NU).astype(np.int64)
        starts = np.searchsorted(u_of, np.arange(NU + 1))
        loads = np.zeros((T, NCHK), np.int64)
        color = np.full(NU, -1, np.int64)
        csize = np.zeros(NCHK, np.int64)
        multi = np.where(tot > 1)[0]
        for u in multi[np.argsort(-tot[multi], kind="stable")]:
            s, e = starts[u], starts[u + 1]
            ts, cs = t_of[s:e], ut_cnt[s:e]
            best, bestkey = 0, None
            for c in range(NCHK):
                over = np.maximum(loads[ts, c] + cs - CELL, 0).sum()
                key = (over, int(np.max(loads[ts, c] + cs)), csize[c])
                if bestkey is None or key < bestkey:
                    best, bestkey = c, key
            color[u] = best
            loads[ts, best] += cs
            csize[best] += 1
        singles = np.where(tot == 1)[0]
        st_t = t_of[starts[singles]]
        for t in range(T):
            su = singles[st_t == t]
            k = len(su)
            if k == 0:
                continue
            l0, l1 = loads[t, 0], loads[t, 1]
            n0 = int(np.clip((k + l1 - l0 + 1) // 2, 0, k))
            color[su[:n0]] = 0
            color[su[n0:]] = 1
            loads[t, 0] += n0
            loads[t, 1] += k - n0
            csize[0] += n0
            csize[1] += k - n0
        assert (color >= 0).all()
        assert csize.max() <= CPAD, (r, csize)
        # row within chunk, in ascending-uniq order (gather locality)
        rowin_u = np.zeros(NU, np.int64)
        for c in range(NCHK):
            sel_u = color == c
            rowin_u[sel_u] = np.arange(int(sel_u.sum()))
        q_e = color[cid]
        rowin_e = rowin_u[cid]
        cnt_tq = np.bincount(t_e * NCHK + q_e, minlength=T * NCHK).reshape(T, NCHK)
        maxslots = max(maxslots, int(cnt_tq.max()))
        percore.append((rl, uniq, q_e, rowin_e, t_e, cnt_tq, color, rowin_u))
    Cq = max(1, -(-maxslots // P))
    cfg.Cq = Cq
    G = cfg.G
    NG = T // G
    IPG = G * Cq * P

    in_maps = []
    for r in range(NC):
        rl, uniq, q_e, rowin_e, t_e, cnt_tq, color, rowin_u = percore[r]
        newpos = cfg.newpos[r]

        # x-compact: chunk q of the table holds source u at column
        # q*CPAD + rowin_u; unused tail columns stay zero.
        xTc = np.zeros((P, NCHK * CPAD), BF16)
        xTc[:, color * CPAD + rowin_u] = xT[:, uniq]

        # order edges by (tile, chunk, SOURCE row) so each gather run reads
        # ascending addresses (HBM row-buffer locality)
        order = np.lexsort((rowin_e, q_e, t_e))
        rl_s, q_s, rw_s, t_s = rl[order], q_e[order], rowin_e[order], t_e[order]
        tq_s = t_s * NCHK + q_s
        run_start = np.zeros(T * NCHK + 1, np.int64)
        np.cumsum(cnt_tq.reshape(-1), out=run_start[1:])
        pos = np.arange(len(rl_s)) - run_start[tq_s]
        tl_s = t_s % G
        gg_s = t_s // G
        ipos = tl_s * (Cq * P) + pos
        idx16 = np.zeros((NCHK, NG, IPG), np.int16)  # pad -> row 0 (sel kills it)
        idx16[q_s, gg_s, ipos] = rw_s.astype(np.int16)
        # wrap each stream: idx i -> [i%16, i//16], replicate to 128 partitions
        idxw = idx16.reshape(NCHK, NG, IPG // 16, 16).transpose(0, 1, 3, 2)
        idxw = np.ascontiguousarray(idxw)
        idxw = np.tile(idxw, (1, 1, 8, 1))           # [NCHK, NG, 128, IPG//16]
        idx_sb = np.ascontiguousarray(
            idxw.transpose(2, 0, 1, 3)).reshape(P, NCHK * NG * (IPG // 16))

        # rowsr: rebased dst row (node % 128) per slot, -1 for pads
        rowsr = np.full((P, T * NCHK * Cq), -1.0, BF16)
        slot_col = t_s * (NCHK * Cq) + q_s * Cq + pos // P
        rowsr[pos % P, slot_col] = (rl_s % P).astype(BF16)

        iotab = np.broadcast_to(
            np.tile(np.arange(P, dtype=BF16)[None, :], (1, NCHK * Cq)),
            (P, NCHK * Cq * P)).copy()

        cnt = np.bincount(rl, minlength=TLP)
        cntb = cnt.astype(np.float32).reshape(T, P).T.copy()
        cntT = cnt.astype(np.float32).reshape(1, TLP)
        degl = np.zeros(TLP, np.float32)
        degl[newpos] = degree[r * NLOC:(r + 1) * NLOC].astype(np.float32)
        degf = degl.reshape(T, P).T.copy()
        degb2 = 1.0 + EPS + EPS * degf
        xTloc = np.zeros((P, TLP), BF16)
        xTloc[:, newpos] = xT[:, r * NLOC:(r + 1) * NLOC]

        in_maps.append({
            "xTc": xTc, "xTloc": xTloc,
            "Wfc": w_fc, "Wcat": wcat,
            "brobF": brobF, "bfc2": bfc2, "lnab": lnab,
            "iotab": iotab, "rowsr": rowsr, "idxs": idx_sb,
            "cntb": cntb, "cntT": cntT, "degf": degf, "degb2": degb2,
            "ident": np.eye(P, dtype=BF16),
        })
    return in_maps


def build(cfg: Cfg):
    """Build the SPMD Bass program (identical on every core)."""
    NC, T, TLP = cfg.NC, cfg.T, cfg.TLP
    NCHK, Cq, CPAD, NCOL = cfg.NCHK, cfg.Cq, cfg.CPAD, cfg.NCOL
    G = cfg.G
    NG = T // G
    IPG = G * Cq * P
    SELW = NCHK * Cq * P       # sel width per tile
    bf = mybir.dt.bfloat16
    f32 = mybir.dt.float32
    f8 = mybir.dt.float8e4
    i16 = mybir.dt.int16
    B = 4                      # tiles per eltwise batch

    nc = bacc.Bacc("TRN2", target_bir_lowering=False, debug=False, num_devices=NC,
                   num_swdge_queues=4)
    # pre-create ACT bias consts so no memsets land mid-loop
    for val in (LN_EPS, 0.0, 1.0):
        if (f32, val) in nc.const_aps.aps:
            continue
        cs = nc.alloc_sbuf_tensor(f"const-float32-{val}", [P, 1], f32)
        nc.gpsimd.memset(cs.ap(), val)
        nc.const_aps.aps[(f32, val)] = cs.ap()
    nc.all_engine_barrier()

    d_xTc = nc.dram_tensor("xTc", [P, NCOL], bf, kind="ExternalInput").ap()
    d_xTloc = nc.dram_tensor("xTloc", [P, TLP], bf, kind="ExternalInput").ap()
    d_wfc = nc.dram_tensor("Wfc", [P, D], bf, kind="ExternalInput").ap()
    d_wcat = nc.dram_tensor("Wcat", [P, 2 * D], bf, kind="ExternalInput").ap()
    d_brobF = nc.dram_tensor("brobF", [P, D], bf, kind="ExternalInput").ap()
    d_bfc2 = nc.dram_tensor("bfc2", [1, D], f32, kind="ExternalInput").ap()
    d_lnab = nc.dram_tensor("lnab", [P, 2 * D], f32, kind="ExternalInput").ap()
    d_iota = nc.dram_tensor("iotab", [P, SELW], bf, kind="ExternalInput").ap()
    d_rowsr = nc.dram_tensor("rowsr", [P, T * NCHK * Cq], bf,
                             kind="ExternalInput").ap()
    d_idxs = nc.dram_tensor("idxs", [P, NCHK * NG * (IPG // 16)], i16,
                            kind="ExternalInput").ap()
    d_cntb = nc.dram_tensor("cntb", [P, T], f32, kind="ExternalInput").ap()
    d_cntT = nc.dram_tensor("cntT", [1, TLP], f32, kind="ExternalInput").ap()
    d_ident = nc.dram_tensor("ident", [P, P], bf, kind="ExternalInput").ap()
    d_degf = nc.dram_tensor("degf", [P, T], f32, kind="ExternalInput").ap()
    d_degb2 = nc.dram_tensor("degb2", [P, T], f32, kind="ExternalInput").ap()
    # one h-table tensor per chunk so chunk-q gathers depend only on chunk-q
    # phase-1 writes
    d_hq = [nc.dram_tensor(f"htab{q}", [CPAD, D], bf, kind="Internal").ap()
            for q in range(NCHK)]
    d_y = nc.dram_tensor("y", [TLP, D], f32, kind="ExternalOutput").ap()

    with tile.TileContext(nc) as tc, ExitStack() as ctx:
        from concourse import library_config
        nc.gpsimd.load_library(library_config.mlp)

        # ------------- consts: load FIRST, on the Scalar queue -------------
        consts = ctx.enter_context(tc.tile_pool(name="consts", bufs=1))
        wfc = consts.tile([P, D], bf)
        nc.scalar.dma_start(wfc[:], d_wfc[:])
        wcat = consts.tile([P, 2 * D], bf)
        nc.scalar.dma_start(wcat[:], d_wcat[:])
        brobF = consts.tile([P, D], bf)
        nc.scalar.dma_start(brobF[:], d_brobF[:])
        bfc2r = consts.tile([1, D], f32)
        nc.scalar.dma_start(bfc2r[:], d_bfc2[:])
        # phase-3-only consts go on the gpsimd DMA queue (idle until the
        # gathers) so they don't delay phase-1 table writes on scalar's queue
        iota = consts.tile([P, SELW], bf)
        nc.gpsimd.dma_start(iota[:], d_iota[:])
        rowsr = consts.tile([P, T * NCHK * Cq], bf)
        nc.gpsimd.dma_start(rowsr[:], d_rowsr[:])
        idxs = consts.tile([P, NCHK * NG * (IPG // 16)], i16)
        nc.gpsimd.dma_start(idxs[:], d_idxs[:])
        cntb = consts.tile([P, T], f32)
        nc.scalar.dma_start(cntb[:], d_cntb[:])
        cntT = consts.tile([1, TLP], f32)
        nc.scalar.dma_start(cntT[:], d_cntT[:])
        degf = consts.tile([P, T], f32)
        nc.scalar.dma_start(degf[:], d_degf[:])
        degb2 = consts.tile([P, T], f32)
        nc.scalar.dma_start(degb2[:], d_degb2[:])
        xloc = consts.tile([P, TLP], bf)
        nc.gpsimd.dma_start(xloc[:], d_xTloc[:])
        ident = consts.tile([P, P], bf)
        nc.gpsimd.dma_start(ident[:], d_ident[:])
        lnab = None
        if not cfg.ln_trivial:
            lnab = consts.tile([P, 2 * D], f32)
            nc.scalar.dma_start(lnab[:], d_lnab[:])

        # ---------------- phase 1: g = x @ W_fc table ----------------
        CHUNK = 8192
        STG = 2048
        GRP = 512
        with tc.tile_pool(name="p1x", bufs=3) as p1x, \
             tc.tile_pool(name="p1ps", bufs=4, space="PSUM") as p1ps, \
             tc.tile_pool(name="p1st", bufs=4) as p1st:
            evac = 0
            for q in range(NCHK):
                for c0 in range(0, CPAD, CHUNK):
                    cw = min(CHUNK, CPAD - c0)
                    xc = p1x.tile([P, CHUNK], bf, tag="xc", name="xc")
                    nc.sync.dma_start(xc[:, :cw],
                                      d_xTc[:, q * CPAD + c0:q * CPAD + c0 + cw])
                    for g0 in range(0, cw, STG):
                        gw = min(STG, cw - g0)
                        gst = p1st.tile([P, STG], bf, tag="gst", name="gst")
                        for s in range(0, gw, GRP):
                            gps = p1ps.tile([P, GRP], f32, space="PSUM",
                                            tag="gps", name="gps")
                            for j in range(0, GRP, P):
                                nc.tensor.matmul(
                                    out=gps[:, j:j + P],
                                    lhsT=xc[:, g0 + s + j:g0 + s + j + P],
                                    rhs=wfc[:],
                                    start=True, stop=True,
                                )
                            if evac % 3 == 2:
                                nc.vector.tensor_scalar_mul(
                                    out=gst[:, s:s + GRP], in0=gps[:],
                                    scalar1=1.0)
                            else:
                                nc.scalar.copy(gst[:, s:s + GRP], gps[:])
                            evac += 1
                        dst = d_hq[q][c0 + g0:c0 + g0 + gw, :].rearrange(
                            "(t p) d -> p t d", p=P)
                        nc.scalar.dma_start(dst, gst[:, :gw].rearrange(
                            "p (t d) -> p t d", d=D))

        # ---------------- phase 3: message passing + elementwise -------------
        msgp = ctx.enter_context(tc.tile_pool(name="msgp", bufs=4))
        selp = ctx.enter_context(tc.tile_pool(name="selp", bufs=5))
        eltp = ctx.enter_context(tc.tile_pool(name="eltp", bufs=2))
        smallp = ctx.enter_context(tc.tile_pool(name="smallp", bufs=2))
        apsp = ctx.enter_context(tc.tile_pool(name="apsp", bufs=2, space="PSUM"))
        rgp = ctx.enter_context(tc.tile_pool(name="rgp", bufs=2, space="PSUM"))
        hp = ctx.enter_context(tc.tile_pool(name="hp", bufs=2, space="PSUM"))

        def elt_pre(bt, tiles):
            """Immediate part: rate + gamma evac on Scalar (inputs ready as
            soon as the batch's rg matmuls retire)."""
            nb = len(tiles)
            rg4, aps4, h4 = bt
            spt = eltp.tile([P, B, D], bf, tag="spt", name="spt")
            rate = eltp.tile([P, B, D], bf, tag="rate", name="rate")
            gamb = eltp.tile([P, B, D], bf, tag="gamb", name="gamb")
            # rate = softplus(x@Wrt) = ln(exp(z)+1); exp/ln share one ACT table
            nc.scalar.activation(out=spt[:, :nb, :], in_=rg4[:, :nb, 0:D],
                                 func=mybir.ActivationFunctionType.Exp)
            nc.scalar.activation(out=rate[:, :nb, :], in_=spt[:, :nb, :],
                                 func=mybir.ActivationFunctionType.Ln,
                                 bias=1.0)
            nc.scalar.copy(gamb[:, :nb, :], rg4[:, :nb, D:2 * D])
            return rate, gamb

        def eltwise(bt, tiles, rate, gamb):
            nb = len(tiles)
            t0g = tiles[0]
            rg4, aps4, h4 = bt
            aggb = eltp.tile([P, B, D], bf, tag="aggb", name="aggb")
            num = eltp.tile([P, B, D], bf, tag="num", name="num")
            den = eltp.tile([P, B, D], f32, tag="den", name="den")
            y0 = eltp.tile([P, B, D], f32, tag="y0", name="y0")
            yf = eltp.tile([P, B, D], f32, tag="yf", name="yf")
            st6 = smallp.tile([P, B, 6], f32, tag="st6", name="st6")
            mv = smallp.tile([P, B, 2], f32, tag="mv", name="mv")
            sm = smallp.tile([P, 2 * B], f32, tag="sm", name="sm")
            lv = sm[:, 0:nb]
            rstd = sm[:, B:B + nb]

            # evacuate agg PSUM -> bf16 SBUF on the Scalar engine
            nc.scalar.copy(aggb[:, :nb, :], aps4[:, :nb, :])
            # den = (rate+EPS)*deg + 1 + EPS, per-partition scale/bias per tile
            for jj in range(nb):
                t = t0g + jj
                nc.scalar.activation(
                    out=den[:, jj, :], in_=rate[:, jj, :],
                    func=mybir.ActivationFunctionType.Identity,
                    scale=degf[:, t:t + 1], bias=degb2[:, t:t + 1])
            nc.vector.reciprocal_approx_fast(out=den[:, :nb, :],
                                             in_=den[:, :nb, :])
            # num = (rate+EPS)*agg + gamma + b_rob
            nc.vector.scalar_tensor_tensor(
                out=num[:, :nb, :], in0=rate[:, :nb, :], scalar=EPS,
                in1=aggb[:, :nb, :],
                op0=mybir.AluOpType.add, op1=mybir.AluOpType.mult)
            nc.vector.tensor_add(out=num[:, :nb, :], in0=num[:, :nb, :],
                                 in1=gamb[:, :nb, :])
            brob_b = brobF[:, None, :].to_broadcast([P, nb, D])
            nc.vector.tensor_add(out=num[:, :nb, :], in0=num[:, :nb, :],
                                 in1=brob_b)
            nc.vector.tensor_mul(out=y0[:, :nb, :], in0=num[:, :nb, :],
                                 in1=den[:, :nb, :])
            # LayerNorm stats via bn_stats/bn_aggr -> (mean, var) per tile
            for jj in range(nb):
                nc.vector.bn_stats(out=st6[:, jj, :], in_=y0[:, jj, :])
                nc.vector.bn_aggr(out=mv[:, jj, :], in_=st6[:, jj, :])
            # rstd = (var+eps)^-0.5 = exp(-0.5*ln(var+eps))
            nc.scalar.activation(out=lv, in_=mv[:, :nb, 1],
                                 func=mybir.ActivationFunctionType.Ln,
                                 bias=LN_EPS)
            nc.scalar.activation(out=rstd, in_=lv,
                                 func=mybir.ActivationFunctionType.Exp,
                                 scale=-0.5)
            # nb2 = -mean*rstd; yf = y0*rstd + nb2 (per-partition ACT)
            nb2 = smallp.tile([P, B], f32, tag="nb2", name="nb2")
            nc.vector.scalar_tensor_tensor(
                out=nb2[:, :nb], in0=mv[:, :nb, 0], scalar=-1.0,
                in1=rstd,
                op0=mybir.AluOpType.mult, op1=mybir.AluOpType.mult)
            for jj in range(nb):
                nc.scalar.activation(
                    out=yf[:, jj, :], in_=y0[:, jj, :],
                    func=mybir.ActivationFunctionType.Identity,
                    scale=rstd[:, jj:jj + 1], bias=nb2[:, jj:jj + 1])
            if lnab is not None:
                yf3 = yf[:, :nb, :]
                lg = lnab[:, 0:D][:, None, :].to_broadcast([P, nb, D])
                lb = lnab[:, D:2 * D][:, None, :].to_broadcast([P, nb, D])
                nc.vector.tensor_mul(out=yf3, in0=yf3, in1=lg)
                nc.vector.tensor_add(out=yf3, in0=yf3, in1=lb)
            # y rows permuted to (p*T + t) so each partition writes one
            # contiguous 512B*nb stripe; unshard() undoes the permutation
            dst = d_y[:].rearrange("(p t) d -> p t d", p=P)[:, t0g:t0g + nb, :]
            nc.sync.dma_start(dst, yf[:, :nb, :])

        bt = None
        pending = None
        GCq = G * Cq
        half = [(0, (GCq + 1) // 2), ((GCq + 1) // 2, GCq // 2)]

        def issue_gather(q, gg):
            mq = msgp.tile([P, GCq * D], bf, tag=f"msg{q}", name=f"msg{q}")
            icol = (q * NG + gg) * (IPG // 16)
            for si, (s0, ns) in enumerate(half):
                sec = mq[:, s0 * D:(s0 + ns) * D]
                nc.gpsimd.dma_gather(
                    out_ap=sec.rearrange("p (s d) -> p s d", d=D),
                    in_ap=d_hq[q][:],
                    idxs_ap=idxs[:, icol + s0 * 8:icol + (s0 + ns) * 8],
                    num_idxs=ns * P,
                    num_idxs_reg=ns * P,
                    elem_size=D,
                    single_packet=False,
                    queue_num=(2 * gg + si + 2 * q) % 4,
                )
            return mq

        for gg in range(NG):
            tg0 = gg * G
            msgc = [issue_gather(0, gg), issue_gather(1, gg)]
            for tl in range(G):
                t = tg0 + tl
                j = t % B
                if j == 0:
                    bt = (rgp.tile([P, B, 2 * D], f32, space="PSUM", tag="rg4",
                                   name="rg4"),
                          apsp.tile([P, B, D], f32, space="PSUM", tag="aps4",
                                    name="aps4"),
                          hp.tile([P, B, D], f32, space="PSUM", tag="h4",
                                  name="h4"))

                rg4, aps4, h4 = bt
                sel = selp.tile([P, SELW], bf, tag="sel", name="sel")
                rb = rowsr[:, t * NCHK * Cq:(t + 1) * NCHK * Cq][:, :, None] \
                    .to_broadcast([P, NCHK * Cq, P])
                nc.vector.tensor_tensor(
                    out=sel.rearrange("p (c m) -> p c m", c=NCHK * Cq), in0=rb,
                    in1=iota.rearrange("p (c m) -> p c m", c=NCHK * Cq),
                    op=mybir.AluOpType.is_equal)
                # fused rate|gamma GEMM (moving N=256)
                nc.tensor.matmul(out=rg4[:, j, :],
                                 lhsT=xloc[:, t * P:(t + 1) * P],
                                 rhs=wcat[:], start=True, stop=True)
                # local h rows for the self term (bias-free)
                nc.tensor.matmul(out=h4[:, j, :],
                                 lhsT=xloc[:, t * P:(t + 1) * P],
                                 rhs=wfc[:], start=True, stop=True)
                hlc = selp.tile([P, D], bf, tag="hlc", name="hlc")
                nc.scalar.activation(out=hlc[:], in_=h4[:, j, :],
                                     func=mybir.ActivationFunctionType.Copy,
                                     scale=cntb[:, t:t + 1])
                # aps = ident@hlc + cntT x 2bfc + sum_sel sel@msg
                nc.tensor.matmul(out=aps4[:, j, :], lhsT=ident[:], rhs=hlc[:],
                                 start=True, stop=False)
                nc.tensor.matmul(out=aps4[:, j, :],
                                 lhsT=cntT[0:1, t * P:(t + 1) * P],
                                 rhs=bfc2r[0:1, :], start=False, stop=False)
                for q in range(NCHK):
                    for c in range(Cq):
                        cc = q * Cq + c
                        moff = (tl * Cq + c) * D
                        nc.tensor.matmul(
                            out=aps4[:, j, :], lhsT=sel[:, cc * P:(cc + 1) * P],
                            rhs=msgc[q][:, moff:moff + D],
                            start=False, stop=(cc == NCHK * Cq - 1))
                if j == B - 1 or t == T - 1:
                    tiles = list(range(t - j, t + 1))
                    rate, gamb = elt_pre(bt, tiles)
                    if pending is not None:
                        eltwise(*pending)
                    pending = (bt, tiles, rate, gamb)
        if pending is not None:
            eltwise(*pending)

    nc.compile()
    return nc


def unshard(res, cfg: Cfg):
    """Gather per-core outputs back to the full array, undoing the y-row
    permutation (row p*T+t holds node t*128+p) and the node->tile shuffle."""
    T = cfg.T
    ys = []
    for r in range(cfg.NC):
        yd = res.results[r]["y"].reshape(P, T, D).transpose(1, 0, 2)
        ys.append(yd.reshape(cfg.TLP, D)[cfg.newpos[r]])
    return np.concatenate(ys, axis=0)


def run(inputs, cfg: Cfg, core_ids=None):
    in_maps = prep(**inputs, cfg=cfg)
    nc = build(cfg)
    res = run_bass_kernel_spmd(nc, in_maps, core_ids=core_ids or list(range(cfg.NC)))
    return unshard(res, cfg)


def kernel(**inputs):
    cfg = Cfg(N=100_000, E=800_000, NC=8)
    return run(inputs, cfg)


# revision 82
# speedup vs baseline: 1.5807x; 1.0019x over previous
"""Trainium2 Bass kernel for a GNN message-passing layer (BoundaryConvLayer).

Computation (reference, per node i over D=128 channels):
    rate  = softplus(x @ W_rate) + EPS
    gamma = x @ W_rob + b_rob
    h     = x @ W_fc + b_fc
    agg   = segment_sum(h[row] + h[col], row)
    y     = LayerNorm((rate*agg + gamma) / (1 + rate*deg + EPS)) * ln_gamma + ln_beta

Distribution: nodes sharded across 8 cores by contiguous row blocks; edges
partitioned by destination row so the segment sum is local.

Design (v5):
  * Per-core COMPACT gather table (phase 1): only the ~63% of nodes referenced
    as sources by this core's edges are materialized (g = x@W_fc rows, bf16),
    split into NCHK=2 chunks so dma_gather's int16 indices reach all rows.
    PSUM evacuation alternates Scalar (2/3) and Vector (1/3).
  * agg identity: agg[i] = cnt[i]*h[i] + sum_{e:row=i} h[col_e], cnt = in-edge
    count.  Neighbor sum via one-hot "selection matrix" matmuls accumulated in
    PSUM; self term via identity matmul of hlc = cnt*g_local; the fc bias
    enters as a K=1 matmul (cntT x 2*b_fc), b_rob as a broadcast DVE add.
  * Per-tile GEMMs fused: one matmul with moving rhs [W_rate|W_rob] (N=256)
    plus one for W_fc.  Eltwise is SOFTWARE-PIPELINED one batch behind the
    matmuls (in-order engines never head-of-line block): rate/gamma evac
    issue immediately at batch end (elt_pre), the rest one batch later.
    den and the final (y-mean)*rstd run as per-partition-scale/bias Scalar
    ACTs; LayerNorm stats via DVE bn_stats/bn_aggr; fast custom-DVE
    reciprocal for 1/den.
"""

import numpy as np
import ml_dtypes
from contextlib import ExitStack
from dataclasses import dataclass

import concourse.bass as bass
import concourse.tile as tile
from concourse import bacc, mybir
from concourse.bass_utils import run_bass_kernel_spmd

# The stock ACT-table chooser greedily picks the first set containing each
# function, which can alternate between sets and reload the table (~1.3us
# each).  Restrict it to the one set that contains all of {Exp, Ln, Copy}.
_ACT_KEEP = "natural_log_exp_and_others"
if not getattr(bacc, "_act_tables_patched", False):
    _orig_get_tables = bacc.get_activation_tables

    def _patched_get_tables(arch):
        t = _orig_get_tables(arch)
        if _ACT_KEEP in t:
            t = {k: (v if k == _ACT_KEEP else set()) for k, v in t.items()}
        return t

    bacc.get_activation_tables = _patched_get_tables
    bacc._act_tables_patched = True

BF16 = ml_dtypes.bfloat16
EPS = 1e-4
LN_EPS = 1e-5
P = 128
D = 128


@dataclass
class Cfg:
    N: int            # total nodes
    E: int            # total edges
    NC: int           # cores
    NCHK: int = 2     # gather table chunks (int16 range)
    CPAD: int = 32256 # rows per chunk (252*128, < 32768 for int16 idx)
    Cq: int = 0       # 128-slot groups per (tile, chunk); set by prep
    ln_trivial: bool = False

    @property
    def NLOC(self):
        return self.N // self.NC

    @property
    def T(self):
        return (self.NLOC + P - 1) // P

    @property
    def TLP(self):
        return self.T * P

    @property
    def NCOL(self):  # x-compact table columns
        return self.NCHK * self.CPAD

    @property
    def G(self):      # tiles per gather group
        for g in (7, 14, 4, 2, 1):
            if self.T % g == 0:
                return g
        return 1


def prep(x, edge_index, degree, W_fc, b_fc, W_rate, W_rob, b_rob, ln_gamma, ln_beta,
         cfg: Cfg):
    """Host-side preprocessing: shard + build per-core compact gather tables."""
    N, NC, NCHK, CPAD = cfg.N, cfg.NC, cfg.NCHK, cfg.CPAD
    NLOC, T, TLP = cfg.NLOC, cfg.T, cfg.TLP

    x = np.asarray(x, np.float32)
    edge_index = np.asarray(edge_index, np.int64)
    degree = np.asarray(degree)
    row, col = edge_index[0], edge_index[1]
    xT = x.T.astype(BF16)  # [D, N]

    w_fc = np.ascontiguousarray(W_fc, dtype=np.float32).astype(BF16)
    w_rt = np.ascontiguousarray(W_rate, dtype=np.float32).astype(BF16)
    w_rb = np.ascontiguousarray(W_rob, dtype=np.float32).astype(BF16)
    wcat = np.concatenate([w_rt, w_rb], axis=1)          # [128, 256]
    # b_rob is folded into the eltwise as a broadcast add (brobF, replicated
    # across partitions); no bias matmul needed for the rate|gamma GEMM.
    brobF = np.broadcast_to(np.asarray(b_rob, np.float32)[None, :],
                            (P, D)).astype(BF16).copy()
    # table is bias-free (g = x@W_fc); the self-term chain adds cnt*2bfc via
    # a K=1 matmul (cntT x bfc2), so
    # cnt*g_i + cnt*2bfc + sum_slots g[col] == cnt*h_i + sum h[col] exactly
    bfc2 = 2.0 * np.asarray(b_fc, np.float32).reshape(1, D)
    onesr = np.ones((1, P), np.float32)

    cfg.ln_trivial = bool(np.all(np.asarray(ln_gamma) == 1.0)
                          and np.all(np.asarray(ln_beta) == 0.0))
    lnab = np.zeros((P, 2 * D), np.float32)
    lnab[:, :D] = np.asarray(ln_gamma, np.float32)[None, :]
    lnab[:, D:] = np.asarray(ln_beta, np.float32)[None, :]

    core_of = row // NLOC
    CELL = 4 * P          # target per-(tile,chunk) occupancy for Cq=4

    # pass 1: per-core tile balancing (permute local nodes so every tile has
    # <=128 nodes and ~<=2*CELL edges) + greedy source 2-coloring so each
    # (tile, chunk) cell stays <= CELL.  If a core misses, Cq grows to 5 and
    # the program adapts (capacity is computed from the achieved maximum).
    import heapq
    percore = []
    maxslots = 0
    cfg.newpos = []
    for r in range(NC):
        m = core_of == r
        rl0 = row[m] - r * NLOC
        ce = col[m]

        # -- node -> tile assignment (LPT greedy on edge count, <=128 nodes)
        cnt_node = np.bincount(rl0, minlength=NLOC)
        order_n = np.argsort(-cnt_node, kind="stable")
        heap = [(0, t) for t in range(T)]
        heapq.heapify(heap)
        nslots = np.zeros(T, np.int64)
        tile_of = np.zeros(NLOC, np.int64)
        for nid in order_n:
            c = cnt_node[nid]
            while True:
                load, t = heapq.heappop(heap)
                if nslots[t] < P:
                    break  # full tiles never take nodes again: drop them
            tile_of[nid] = t
            nslots[t] += 1
            heapq.heappush(heap, (load + int(c), t))
        # slot within tile
        slot_in = np.zeros(NLOC, np.int64)
        fill = np.zeros(T, np.int64)
        for nid in np.argsort(tile_of, kind="stable"):
            t = tile_of[nid]
            slot_in[nid] = fill[t]
            fill[t] += 1
        newpos = tile_of * P + slot_in          # orig local id -> device row
        cfg.newpos.append(newpos)
        rl = newpos[rl0]
        t_e = rl // P

        # -- source chunk 2-coloring
        uniq, cid = np.unique(ce, return_inverse=True)
        NU = len(uniq)
        ut, ut_cnt = np.unique(cid * T + t_e, return_counts=True)
        u_of = ut // T
        t_of = ut % T
        tot = np.bincount(u_of, weights=ut_cnt, minlength=NU).astype(np.int64)
        starts = np.searchsorted(u_of, np.arange(NU + 1))
        loads = np.zeros((T, NCHK), np.int64)
        color = np.full(NU, -1, np.int64)
        csize = np.zeros(NCHK, np.int64)
        multi = np.where(tot > 1)[0]
        for u in multi[np.argsort(-tot[multi], kind="stable")]:
            s, e = starts[u], starts[u + 1]
            ts, cs = t_of[s:e], ut_cnt[s:e]
            best, bestkey = 0, None
            for c in range(NCHK):
                over = np.maximum(loads[ts, c] + cs - CELL, 0).sum()
                key = (over, int(np.max(loads[ts, c] + cs)), csize[c])
                if bestkey is None or key < bestkey:
                    best, bestkey = c, key
            color[u] = best
            loads[ts, best] += cs
            csize[best] += 1
        singles = np.where(tot == 1)[0]
        st_t = t_of[starts[singles]]
        for t in range(T):
            su = singles[st_t == t]
            k = len(su)
            if k == 0:
                continue
            l0, l1 = loads[t, 0], loads[t, 1]
            n0 = int(np.clip((k + l1 - l0 + 1) // 2, 0, k))
            color[su[:n0]] = 0
            color[su[n0:]] = 1
            loads[t, 0] += n0
            loads[t, 1] += k - n0
            csize[0] += n0
            csize[1] += k - n0
        assert (color >= 0).all()
        assert csize.max() <= CPAD, (r, csize)
        # row within chunk, in ascending-uniq order (gather locality)
        rowin_u = np.zeros(NU, np.int64)
        for c in range(NCHK):
            sel_u = color == c
            rowin_u[sel_u] = np.arange(int(sel_u.sum()))
        q_e = color[cid]
        rowin_e = rowin_u[cid]
        cnt_tq = np.bincount(t_e * NCHK + q_e, minlength=T * NCHK).reshape(T, NCHK)
        maxslots = max(maxslots, int(cnt_tq.max()))
        percore.append((rl, uniq, q_e, rowin_e, t_e, cnt_tq, color, rowin_u))
    Cq = max(1, -(-maxslots // P))
    cfg.Cq = Cq
    G = cfg.G
    NG = T // G
    IPG = G * Cq * P

    in_maps = []
    for r in range(NC):
        rl, uniq, q_e, rowin_e, t_e, cnt_tq, color, rowin_u = percore[r]
        newpos = cfg.newpos[r]

        # x-compact: chunk q of the table holds source u at column
        # q*CPAD + rowin_u; unused tail columns stay zero.
        xTc = np.zeros((P, NCHK * CPAD), BF16)
        xTc[:, color * CPAD + rowin_u] = xT[:, uniq]

        # order edges by (tile, chunk, SOURCE row) so each gather run reads
        # ascending addresses (HBM row-buffer locality)
        order = np.lexsort((rowin_e, q_e, t_e))
        rl_s, q_s, rw_s, t_s = rl[order], q_e[order], rowin_e[order], t_e[order]
        tq_s = t_s * NCHK + q_s
        run_start = np.zeros(T * NCHK + 1, np.int64)
        np.cumsum(cnt_tq.reshape(-1), out=run_start[1:])
        pos = np.arange(len(rl_s)) - run_start[tq_s]
        tl_s = t_s % G
        gg_s = t_s // G
        ipos = tl_s * (Cq * P) + pos
        idx16 = np.zeros((NCHK, NG, IPG), np.int16)  # pad -> row 0 (sel kills it)
        idx16[q_s, gg_s, ipos] = rw_s.astype(np.int16)
        # wrap each stream: idx i -> [i%16, i//16], replicate to 128 partitions
        idxw = idx16.reshape(NCHK, NG, IPG // 16, 16).transpose(0, 1, 3, 2)
        idxw = np.ascontiguousarray(idxw)
        idxw = np.tile(idxw, (1, 1, 8, 1))           # [NCHK, NG, 128, IPG//16]
        idx_sb = np.ascontiguousarray(
            idxw.transpose(2, 0, 1, 3)).reshape(P, NCHK * NG * (IPG // 16))

        # rowsr: rebased dst row (node % 128) per slot, -1 for pads
        rowsr = np.full((P, T * NCHK * Cq), -1.0, BF16)
        slot_col = t_s * (NCHK * Cq) + q_s * Cq + pos // P
        rowsr[pos % P, slot_col] = (rl_s % P).astype(BF16)

        iotab = np.broadcast_to(
            np.tile(np.arange(P, dtype=BF16)[None, :], (1, NCHK * Cq)),
            (P, NCHK * Cq * P)).copy()

        cnt = np.bincount(rl, minlength=TLP)
        cntb = cnt.astype(np.float32).reshape(T, P).T.copy()
        cntT = cnt.astype(np.float32).reshape(1, TLP)
        degl = np.zeros(TLP, np.float32)
        degl[newpos] = degree[r * NLOC:(r + 1) * NLOC].astype(np.float32)
        degf = degl.reshape(T, P).T.copy()
        degb2 = 1.0 + EPS + EPS * degf
        xTloc = np.zeros((P, TLP), BF16)
        xTloc[:, newpos] = xT[:, r * NLOC:(r + 1) * NLOC]

        in_maps.append({
            "xTc": xTc, "xTloc": xTloc,
            "Wfc": w_fc, "Wcat": wcat,
            "brobF": brobF, "bfc2": bfc2, "lnab": lnab,
            "iotab": iotab, "rowsr": rowsr, "idxs": idx_sb,
            "cntb": cntb, "cntT": cntT, "degf": degf, "degb2": degb2,
            "ident": np.eye(P, dtype=BF16),
        })
    return in_maps


def build(cfg: Cfg):
    """Build the SPMD Bass program (identical on every core)."""
    NC, T, TLP = cfg.NC, cfg.T, cfg.TLP
    NCHK, Cq, CPAD, NCOL = cfg.NCHK, cfg.Cq, cfg.CPAD, cfg.NCOL
    G = cfg.G
    NG = T // G
    IPG = G * Cq * P
    SELW = NCHK * Cq * P       # sel width per tile
    bf = mybir.dt.bfloat16
    f32 = mybir.dt.float32
    f8 = mybir.dt.float8e4
    i16 = mybir.dt.int16
    B = 4                      # tiles per eltwise batch

    nc = bacc.Bacc("TRN2", target_bir_lowering=False, debug=False, num_devices=NC,
                   num_swdge_queues=4)
    # pre-create ACT bias consts so no memsets land mid-loop
    for val in (LN_EPS, 0.0, 1.0):
        if (f32, val) in nc.const_aps.aps:
            continue
        cs = nc.alloc_sbuf_tensor(f"const-float32-{val}", [P, 1], f32)
        nc.gpsimd.memset(cs.ap(), val)
        nc.const_aps.aps[(f32, val)] = cs.ap()
    nc.all_engine_barrier()

    d_xTc = nc.dram_tensor("xTc", [P, NCOL], bf, kind="ExternalInput").ap()
    d_xTloc = nc.dram_tensor("xTloc", [P, TLP], bf, kind="ExternalInput").ap()
    d_wfc = nc.dram_tensor("Wfc", [P, D], bf, kind="ExternalInput").ap()
    d_wcat = nc.dram_tensor("Wcat", [P, 2 * D], bf, kind="ExternalInput").ap()
    d_brobF = nc.dram_tensor("brobF", [P, D], bf, kind="ExternalInput").ap()
    d_bfc2 = nc.dram_tensor("bfc2", [1, D], f32, kind="ExternalInput").ap()
    d_lnab = nc.dram_tensor("lnab", [P, 2 * D], f32, kind="ExternalInput").ap()
    d_iota = nc.dram_tensor("iotab", [P, SELW], bf, kind="ExternalInput").ap()
    d_rowsr = nc.dram_tensor("rowsr", [P, T * NCHK * Cq], bf,
                             kind="ExternalInput").ap()
    d_idxs = nc.dram_tensor("idxs", [P, NCHK * NG * (IPG // 16)], i16,
                            kind="ExternalInput").ap()
    d_cntb = nc.dram_tensor("cntb", [P, T], f32, kind="ExternalInput").ap()
    d_cntT = nc.dram_tensor("cntT", [1, TLP], f32, kind="ExternalInput").ap()
    d_ident = nc.dram_tensor("ident", [P, P], bf, kind="ExternalInput").ap()
    d_degf = nc.dram_tensor("degf", [P, T], f32, kind="ExternalInput").ap()
    d_degb2 = nc.dram_tensor("degb2", [P, T], f32, kind="ExternalInput").ap()
    # one h-table tensor per chunk so chunk-q gathers depend only on chunk-q
    # phase-1 writes
    d_hq = [nc.dram_tensor(f"htab{q}", [CPAD, D], bf, kind="Internal").ap()
            for q in range(NCHK)]
    d_y = nc.dram_tensor("y", [TLP, D], f32, kind="ExternalOutput").ap()

    with tile.TileContext(nc) as tc, ExitStack() as ctx:
        from concourse import library_config
        nc.gpsimd.load_library(library_config.mlp)

        # ------------- consts: load FIRST, on the Scalar queue -------------
        consts = ctx.enter_context(tc.tile_pool(name="consts", bufs=1))
        wfc = consts.tile([P, D], bf)
        nc.scalar.dma_start(wfc[:], d_wfc[:])
        wcat = consts.tile([P, 2 * D], bf)
        nc.scalar.dma_start(wcat[:], d_wcat[:])
        brobF = consts.tile([P, D], bf)
        nc.scalar.dma_start(brobF[:], d_brobF[:])
        bfc2r = consts.tile([1, D], f32)
        nc.scalar.dma_start(bfc2r[:], d_bfc2[:])
        # phase-3-only consts go on the gpsimd DMA queue (idle until the
        # gathers) so they don't delay phase-1 table writes on scalar's queue
        iota = consts.tile([P, SELW], bf)
        nc.gpsimd.dma_start(iota[:], d_iota[:])
        rowsr = consts.tile([P, T * NCHK * Cq], bf)
        nc.gpsimd.dma_start(rowsr[:], d_rowsr[:])
        idxs = consts.tile([P, NCHK * NG * (IPG // 16)], i16)
        nc.gpsimd.dma_start(idxs[:], d_idxs[:])
        cntb = consts.tile([P, T], f32)
        nc.scalar.dma_start(cntb[:], d_cntb[:])
        cntT = consts.tile([1, TLP], f32)
        nc.scalar.dma_start(cntT[:], d_cntT[:])
        degf = consts.tile([P, T], f32)
        nc.scalar.dma_start(degf[:], d_degf[:])
        degb2 = consts.tile([P, T], f32)
        nc.scalar.dma_start(degb2[:], d_degb2[:])
        xloc = consts.tile([P, TLP], bf)
        nc.gpsimd.dma_start(xloc[:], d_xTloc[:])
        ident = consts.tile([P, P], bf)
        nc.gpsimd.dma_start(ident[:], d_ident[:])
        lnab = None
        if not cfg.ln_trivial:
            lnab = consts.tile([P, 2 * D], f32)
            nc.scalar.dma_start(lnab[:], d_lnab[:])

        # ---------------- phase 1: g = x @ W_fc table ----------------
        CHUNK = 8192
        STG = 2048
        GRP = 512
        with tc.tile_pool(name="p1x", bufs=4) as p1x, \
             tc.tile_pool(name="p1ps", bufs=6, space="PSUM") as p1ps, \
             tc.tile_pool(name="p1st", bufs=6) as p1st:
            evac = 0
            for q in range(NCHK):
                for c0 in range(0, CPAD, CHUNK):
                    cw = min(CHUNK, CPAD - c0)
                    xc = p1x.tile([P, CHUNK], bf, tag="xc", name="xc")
                    nc.sync.dma_start(xc[:, :cw],
                                      d_xTc[:, q * CPAD + c0:q * CPAD + c0 + cw])
                    for g0 in range(0, cw, STG):
                        gw = min(STG, cw - g0)
                        gst = p1st.tile([P, STG], bf, tag="gst", name="gst")
                        for s in range(0, gw, GRP):
                            gps = p1ps.tile([P, GRP], f32, space="PSUM",
                                            tag="gps", name="gps")
                            for j in range(0, GRP, P):
                                nc.tensor.matmul(
                                    out=gps[:, j:j + P],
                                    lhsT=xc[:, g0 + s + j:g0 + s + j + P],
                                    rhs=wfc[:],
                                    start=True, stop=True,
                                )
                            if evac % 3 == 2:
                                nc.vector.tensor_scalar_mul(
                                    out=gst[:, s:s + GRP], in0=gps[:],
                                    scalar1=1.0)
                            else:
                                nc.scalar.copy(gst[:, s:s + GRP], gps[:])
                            evac += 1
                        dst = d_hq[q][c0 + g0:c0 + g0 + gw, :].rearrange(
                            "(t p) d -> p t d", p=P)
                        nc.scalar.dma_start(dst, gst[:, :gw].rearrange(
                            "p (t d) -> p t d", d=D))

        # ---------------- phase 3: message passing + elementwise -------------
        msgp = ctx.enter_context(tc.tile_pool(name="msgp", bufs=4))
        selp = ctx.enter_context(tc.tile_pool(name="selp", bufs=5))
        eltp = ctx.enter_context(tc.tile_pool(name="eltp", bufs=2))
        smallp = ctx.enter_context(tc.tile_pool(name="smallp", bufs=2))
        apsp = ctx.enter_context(tc.tile_pool(name="apsp", bufs=2, space="PSUM"))
        rgp = ctx.enter_context(tc.tile_pool(name="rgp", bufs=2, space="PSUM"))
        hp = ctx.enter_context(tc.tile_pool(name="hp", bufs=2, space="PSUM"))

        def elt_pre(bt, tiles):
            """Immediate part: rate + gamma evac on Scalar (inputs ready as
            soon as the batch's rg matmuls retire)."""
            nb = len(tiles)
            rg4, aps4, h4 = bt
            spt = eltp.tile([P, B, D], bf, tag="spt", name="spt")
            rate = eltp.tile([P, B, D], bf, tag="rate", name="rate")
            gamb = eltp.tile([P, B, D], bf, tag="gamb", name="gamb")
            # rate = softplus(x@Wrt) = ln(exp(z)+1); exp/ln share one ACT table
            nc.scalar.activation(out=spt[:, :nb, :], in_=rg4[:, :nb, 0:D],
                                 func=mybir.ActivationFunctionType.Exp)
            nc.scalar.activation(out=rate[:, :nb, :], in_=spt[:, :nb, :],
                                 func=mybir.ActivationFunctionType.Ln,
                                 bias=1.0)
            nc.scalar.copy(gamb[:, :nb, :], rg4[:, :nb, D:2 * D])
            return rate, gamb

        def eltwise(bt, tiles, rate, gamb):
            nb = len(tiles)
            t0g = tiles[0]
            rg4, aps4, h4 = bt
            aggb = eltp.tile([P, B, D], bf, tag="aggb", name="aggb")
            num = eltp.tile([P, B, D], bf, tag="num", name="num")
            den = eltp.tile([P, B, D], f32, tag="den", name="den")
            y0 = eltp.tile([P, B, D], bf, tag="y0", name="y0")
            yf = eltp.tile([P, B, D], f32, tag="yf", name="yf")
            st6 = smallp.tile([P, B, 6], f32, tag="st6", name="st6")
            mv = smallp.tile([P, B, 2], f32, tag="mv", name="mv")
            sm = smallp.tile([P, 2 * B], f32, tag="sm", name="sm")
            lv = sm[:, 0:nb]
            rstd = sm[:, B:B + nb]

            # evacuate agg PSUM -> bf16 SBUF on the Scalar engine
            nc.scalar.copy(aggb[:, :nb, :], aps4[:, :nb, :])
            # den = (rate+EPS)*deg + 1 + EPS, per-partition scale/bias per tile
            for jj in range(nb):
                t = t0g + jj
                nc.scalar.activation(
                    out=den[:, jj, :], in_=rate[:, jj, :],
                    func=mybir.ActivationFunctionType.Identity,
                    scale=degf[:, t:t + 1], bias=degb2[:, t:t + 1])
            nc.vector.reciprocal_approx_fast(out=den[:, :nb, :],
                                             in_=den[:, :nb, :])
            # num = (rate+EPS)*agg + gamma + b_rob
            nc.vector.scalar_tensor_tensor(
                out=num[:, :nb, :], in0=rate[:, :nb, :], scalar=EPS,
                in1=aggb[:, :nb, :],
                op0=mybir.AluOpType.add, op1=mybir.AluOpType.mult)
            nc.vector.tensor_add(out=num[:, :nb, :], in0=num[:, :nb, :],
                                 in1=gamb[:, :nb, :])
            brob_b = brobF[:, None, :].to_broadcast([P, nb, D])
            nc.vector.tensor_add(out=num[:, :nb, :], in0=num[:, :nb, :],
                                 in1=brob_b)
            nc.vector.tensor_mul(out=y0[:, :nb, :], in0=num[:, :nb, :],
                                 in1=den[:, :nb, :])
            # LayerNorm stats via bn_stats/bn_aggr -> (mean, var) per tile
            for jj in range(nb):
                nc.vector.bn_stats(out=st6[:, jj, :], in_=y0[:, jj, :])
                nc.vector.bn_aggr(out=mv[:, jj, :], in_=st6[:, jj, :])
            # rstd = (var+eps)^-0.5 = exp(-0.5*ln(var+eps))
            nc.scalar.activation(out=lv, in_=mv[:, :nb, 1],
                                 func=mybir.ActivationFunctionType.Ln,
                                 bias=LN_EPS)
            nc.scalar.activation(out=rstd, in_=lv,
                                 func=mybir.ActivationFunctionType.Exp,
                                 scale=-0.5)
            # nb2 = -mean*rstd; yf = y0*rstd + nb2 (per-partition ACT)
            nb2 = smallp.tile([P, B], f32, tag="nb2", name="nb2")
            nc.vector.scalar_tensor_tensor(
                out=nb2[:, :nb], in0=mv[:, :nb, 0], scalar=-1.0,
                in1=rstd,
                op0=mybir.AluOpType.mult, op1=mybir.AluOpType.mult)
            for jj in range(nb):
                nc.scalar.activation(
                    out=yf[:, jj, :], in_=y0[:, jj, :],
                    func=mybir.ActivationFunctionType.Identity,
                    scale=rstd[:, jj:jj + 1], bias=nb2[:, jj:jj + 1])
            if lnab is not None:
                yf3 = yf[:, :nb, :]
                lg = lnab[:, 0:D][:, None, :].to_broadcast([P, nb, D])
                lb = lnab[:, D:2 * D][:, None, :].to_broadcast([P, nb, D])
                nc.vector.tensor_mul(out=yf3, in0=yf3, in1=lg)
                nc.vector.tensor_add(out=yf3, in0=yf3, in1=lb)
            # y rows permuted to (p*T + t) so each partition writes one
            # contiguous 512B*nb stripe; unshard() undoes the permutation
            dst = d_y[:].rearrange("(p t) d -> p t d", p=P)[:, t0g:t0g + nb, :]
            nc.sync.dma_start(dst, yf[:, :nb, :])

        bt = None
        pending = None
        GCq = G * Cq
        half = [(0, (GCq + 1) // 2), ((GCq + 1) // 2, GCq // 2)]

        def issue_gather(q, gg):
            mq = msgp.tile([P, GCq * D], bf, tag=f"msg{q}", name=f"msg{q}")
            icol = (q * NG + gg) * (IPG // 16)
            for si, (s0, ns) in enumerate(half):
                sec = mq[:, s0 * D:(s0 + ns) * D]
                nc.gpsimd.dma_gather(
                    out_ap=sec.rearrange("p (s d) -> p s d", d=D),
                    in_ap=d_hq[q][:],
                    idxs_ap=idxs[:, icol + s0 * 8:icol + (s0 + ns) * 8],
                    num_idxs=ns * P,
                    num_idxs_reg=ns * P,
                    elem_size=D,
                    single_packet=False,
                    queue_num=(2 * gg + si + 2 * q) % 4,
                )
            return mq

        for gg in range(NG):
            tg0 = gg * G
            msgc = [issue_gather(0, gg), issue_gather(1, gg)]
            for tl in range(G):
                t = tg0 + tl
                j = t % B
                if j == 0:
                    bt = (rgp.tile([P, B, 2 * D], f32, space="PSUM", tag="rg4",
                                   name="rg4"),
                          apsp.tile([P, B, D], f32, space="PSUM", tag="aps4",
                                    name="aps4"),
                          hp.tile([P, B, D], f32, space="PSUM", tag="h4",
                                  name="h4"))

                rg4, aps4, h4 = bt
                sel = selp.tile([P, SELW], bf, tag="sel", name="sel")
                rb = rowsr[:, t * NCHK * Cq:(t + 1) * NCHK * Cq][:, :, None] \
                    .to_broadcast([P, NCHK * Cq, P])
                nc.vector.tensor_tensor(
                    out=sel.rearrange("p (c m) -> p c m", c=NCHK * Cq), in0=rb,
                    in1=iota.rearrange("p (c m) -> p c m", c=NCHK * Cq),
                    op=mybir.AluOpType.is_equal)
                # fused rate|gamma GEMM (moving N=256)
                nc.tensor.matmul(out=rg4[:, j, :],
                                 lhsT=xloc[:, t * P:(t + 1) * P],
                                 rhs=wcat[:], start=True, stop=True)
                # local h rows for the self term (bias-free)
                nc.tensor.matmul(out=h4[:, j, :],
                                 lhsT=xloc[:, t * P:(t + 1) * P],
                                 rhs=wfc[:], start=True, stop=True)
                hlc = selp.tile([P, D], bf, tag="hlc", name="hlc")
                nc.scalar.activation(out=hlc[:], in_=h4[:, j, :],
                                     func=mybir.ActivationFunctionType.Copy,
                                     scale=cntb[:, t:t + 1])
                # aps = ident@hlc + cntT x 2bfc + sum_sel sel@msg
                nc.tensor.matmul(out=aps4[:, j, :], lhsT=ident[:], rhs=hlc[:],
                                 start=True, stop=False)
                nc.tensor.matmul(out=aps4[:, j, :],
                                 lhsT=cntT[0:1, t * P:(t + 1) * P],
                                 rhs=bfc2r[0:1, :], start=False, stop=False)
                for q in range(NCHK):
                    for c in range(Cq):
                        cc = q * Cq + c
                        moff = (tl * Cq + c) * D
                        nc.tensor.matmul(
                            out=aps4[:, j, :], lhsT=sel[:, cc * P:(cc + 1) * P],
                            rhs=msgc[q][:, moff:moff + D],
                            start=False, stop=(cc == NCHK * Cq - 1))
                if j == B - 1 or t == T - 1:
                    tiles = list(range(t - j, t + 1))
                    rate, gamb = elt_pre(bt, tiles)
                    if pending is not None:
                        eltwise(*pending)
                    pending = (bt, tiles, rate, gamb)
        if pending is not None:
            eltwise(*pending)

    nc.compile()
    return nc


def unshard(res, cfg: Cfg):
    """Gather per-core outputs back to the full array, undoing the y-row
    permutation (row p*T+t holds node t*128+p) and the node->tile shuffle."""
    T = cfg.T
    ys = []
    for r in range(cfg.NC):
        yd = res.results[r]["y"].reshape(P, T, D).transpose(1, 0, 2)
        ys.append(yd.reshape(cfg.TLP, D)[cfg.newpos[r]])
    return np.concatenate(ys, axis=0)


def run(inputs, cfg: Cfg, core_ids=None):
    in_maps = prep(**inputs, cfg=cfg)
    nc = build(cfg)
    res = run_bass_kernel_spmd(nc, in_maps, core_ids=core_ids or list(range(cfg.NC)))
    return unshard(res, cfg)


def kernel(**inputs):
    cfg = Cfg(N=100_000, E=800_000, NC=8)
    return run(inputs, cfg)
